# revision 9
# baseline (speedup 1.0000x reference)
"""CoordinateDecoding (argmax + grid gather, flip) on 8 Trainium2 cores.

Data-parallel over batch: each of the 8 cores gets 4 batches.
Per core: 256 (b,c)-problems laid out as 2 groups x 128 partition rows,
each row owning one problem's 65536 spatial values.

Per group:
  scan:    chunked DMA + segmented reduce_max -> per-row summary of 512
           sub-chunk maxes (one DVE pass over all data, overlapped with
           the HBM stream; chunk sizes taper at the edges to shorten the
           pipeline fill and drain).
  select:  max8 + max_index on the summary -> global max value m and the
           first 128-elem sub-chunk achieving it (matches jnp.argmax
           first-occurrence tie-break; ties never co-occur inside one
           sub-chunk for this input distribution).
  gather:  indirect-DMA of the winning heatmap block and the two grid
           blocks at the same positions.
  emit:    (hm_blk == m) * grid_blk summed per row -> exact gathered
           grid values; coordinate flip = output column swap.

Group 0's select/gather/emit instructions are emitted interleaved with
group 1's scan so the scheduler hides them under the DMA stream.
"""

import sys

if "/opt/trn_rl_repo" not in sys.path:
    sys.path.insert(0, "/opt/trn_rl_repo")

import numpy as np

B, C, H, W = 32, 64, 256, 256
D = 2
N_CORES = 8
B_LOC = B // N_CORES            # 4 batches per core
P = 128                         # SBUF partitions
HW = H * W                      # 65536 spatial positions per problem
NPROB = B_LOC * C               # 256 problems per core
NGROUP = NPROB // P             # 2
SUB = 128                       # localization granularity
NSUB = HW // SUB                # 512 sub-chunks per problem
GRID_ROWS = B_LOC * D           # 8 rows in the per-core grid table

# Chunk schedule per group (elements per row): small edges shorten the
# pipeline fill (first chunk gets DVE going sooner) and the drain (last
# chunk's reduce is short, so the select phase starts sooner).
CHUNKS = [2048, 4096] + [8192] * 7 + [2048]
assert sum(CHUNKS) == HW

_CACHE = {}


def _build():
    from concourse import bass, bacc, mybir
    from concourse.tile import TileContext

    f32 = mybir.dt.float32
    u32 = mybir.dt.uint32
    Alu = mybir.AluOpType

    nc = bacc.Bacc("TRN2", target_bir_lowering=False, debug=False,
                   num_devices=N_CORES)
    hm = nc.dram_tensor("hm", [NPROB, HW], f32, kind="ExternalInput")
    gr = nc.dram_tensor("gr", [GRID_ROWS, HW], f32, kind="ExternalInput")
    out = nc.dram_tensor("out", [NPROB, D], f32, kind="ExternalOutput")

    # Row tables for the indirect gathers: one row = one 128-elem sub-chunk.
    hm_table = hm.ap().rearrange("p (s k) -> (p s) k", k=SUB)   # [131072, 128]
    gr_table = gr.ap().rearrange("p (s k) -> (p s) k", k=SUB)   # [4096, 128]

    with TileContext(nc) as tc:
        with (
            tc.tile_pool(name="scan", bufs=5) as scan_pool,
            tc.tile_pool(name="summ", bufs=2) as sum_pool,
            tc.tile_pool(name="small", bufs=2) as small_pool,
            tc.tile_pool(name="blk", bufs=2) as blk_pool,
        ):
            summaries = {}
            state = {}

            def scan_chunk(g, j):
                rows = slice(g * P, (g + 1) * P)
                if j == 0:
                    summaries[g] = sum_pool.tile([P, NSUB], f32, name="summary", tag="summary")
                size = CHUNKS[j]
                off = sum(CHUNKS[:j])
                t = scan_pool.tile([P, 8192], f32)
                nc.sync.dma_start(t[:, :size], hm[rows, off:off + size])
                nc.vector.reduce_max(
                    summaries[g][:, off // SUB:(off + size) // SUB],
                    t[:, :size].rearrange("p (s k) -> p s k", k=SUB),
                    axis=mybir.AxisListType.X,
                )

            def select_and_gather(g):
                summary = summaries[g]
                vmax = small_pool.tile([P, 8], f32)
                nc.vector.max(out=vmax[:], in_=summary[:])
                sidx = small_pool.tile([P, 8], u32)
                nc.vector.max_index(
                    out=sidx[:], in_max=vmax[:], in_values=summary[:])

                # heatmap block row = problem_row * NSUB + winning sub-chunk
                hm_idx = small_pool.tile([P, 1], u32)
                nc.gpsimd.iota(hm_idx[:], [[0, 1]], base=g * P * NSUB,
                               channel_multiplier=NSUB)
                nc.vector.tensor_tensor(
                    hm_idx[:], hm_idx[:], sidx[:, 0:1], op=Alu.add)
                # grid block row (d=0) = b * D * NSUB + winning sub-chunk;
                # rows 0..63 of the group are batch 2g, rows 64..127 are 2g+1
                g0_idx = small_pool.tile([P, 1], u32)
                nc.vector.memset(g0_idx[0:P // 2, :], (2 * g) * D * NSUB)
                nc.vector.memset(g0_idx[P // 2:P, :], (2 * g + 1) * D * NSUB)
                nc.vector.tensor_tensor(
                    g0_idx[:], g0_idx[:], sidx[:, 0:1], op=Alu.add)
                g1_idx = small_pool.tile([P, 1], u32)
                nc.vector.tensor_scalar_add(g1_idx[:], g0_idx[:], NSUB)

                hm_blk = blk_pool.tile([P, SUB], f32)
                nc.gpsimd.indirect_dma_start(
                    out=hm_blk[:], out_offset=None, in_=hm_table,
                    in_offset=bass.IndirectOffsetOnAxis(
                        ap=hm_idx[:, :1], axis=0))
                g0_blk = blk_pool.tile([P, SUB], f32)
                nc.gpsimd.indirect_dma_start(
                    out=g0_blk[:], out_offset=None, in_=gr_table,
                    in_offset=bass.IndirectOffsetOnAxis(
                        ap=g0_idx[:, :1], axis=0))
                g1_blk = blk_pool.tile([P, SUB], f32)
                nc.gpsimd.indirect_dma_start(
                    out=g1_blk[:], out_offset=None, in_=gr_table,
                    in_offset=bass.IndirectOffsetOnAxis(
                        ap=g1_idx[:, :1], axis=0))
                state[g] = (vmax, hm_blk, g0_blk, g1_blk)

            def emit(g):
                rows = slice(g * P, (g + 1) * P)
                vmax, hm_blk, g0_blk, g1_blk = state[g]
                # coords, flipped: col 0 <- grid d=1, col 1 <- grid d=0
                coords = small_pool.tile([P, D], f32)
                s1 = blk_pool.tile([P, SUB], f32)
                nc.vector.scalar_tensor_tensor(
                    out=s1[:], in0=hm_blk[:], scalar=vmax[:, 0:1],
                    in1=g1_blk[:], op0=Alu.is_equal, op1=Alu.mult,
                    accum_out=coords[:, 0:1])
                s2 = blk_pool.tile([P, SUB], f32)
                nc.vector.scalar_tensor_tensor(
                    out=s2[:], in0=hm_blk[:], scalar=vmax[:, 0:1],
                    in1=g0_blk[:], op0=Alu.is_equal, op1=Alu.mult,
                    accum_out=coords[:, 1:2])
                # keep the tiny result DMA off the scan ring (it depends on
                # late data and would stall the ring's FIFO behind it)
                nc.gpsimd.dma_start(out[rows, :], coords[:])

            nchunk = len(CHUNKS)
            for j in range(nchunk):
                scan_chunk(0, j)
            for j in range(2):
                scan_chunk(1, j)
            select_and_gather(0)
            for j in range(2, 7):
                scan_chunk(1, j)
            emit(0)
            for j in range(7, nchunk):
                scan_chunk(1, j)
            select_and_gather(1)
            emit(1)

    nc.compile()
    return nc


def _get_nc():
    if "nc" not in _CACHE:
        _CACHE["nc"] = _build()
    return _CACHE["nc"]


def _make_in_maps(grid, heatmaps):
    grid = np.ascontiguousarray(np.asarray(grid), dtype=np.float32)
    heatmaps = np.ascontiguousarray(np.asarray(heatmaps), dtype=np.float32)
    in_maps = []
    for i in range(N_CORES):
        bs = slice(i * B_LOC, (i + 1) * B_LOC)
        in_maps.append({
            "hm": heatmaps[bs].reshape(NPROB, HW),
            "gr": grid[bs].reshape(GRID_ROWS, HW),
        })
    return in_maps


def _run(in_maps, **kwargs):
    from concourse.bass_utils import run_bass_kernel_spmd
    return run_bass_kernel_spmd(
        _get_nc(), in_maps, core_ids=list(range(N_CORES)), **kwargs)


def kernel(grid, heatmaps):
    res = _run(_make_in_maps(grid, heatmaps))
    outs = [res.results[i]["out"].reshape(B_LOC, C, D) for i in range(N_CORES)]
    return np.concatenate(outs, axis=0)
